# revision 1
# baseline (speedup 1.0000x reference)
"""Causal self-attention with RoPE on 8 Trainium2 NeuronCores.

Sharding: tensor-parallel over heads (4 groups of 4 heads) x data-parallel
over batch (2), one (batch, head-group) pair per core. Each core computes
its heads' QKV projection, RoPE, causal attention, and a row-slice of the
output projection; the host sums the 4 partial projections per batch.

Attention computes scores transposed (k on partitions, q on the free dim,
512-wide q-groups): softmax rowsums come from a ones-vector matmul, the
probabilities feed P@V directly as the moving operand, and no per-block
transposes of the probability matrix are needed.

Matmul operands use float32r (~1.7 cycles/col at N=512 vs ~5 for float32,
measured); accumulation stays fp32 in PSUM. End-to-end relative error vs
the fp32 reference is ~3.5e-4.

Hardcoded problem shape: x (2,2048,2048), Wqkv (2048,6144), Wproj
(2048,2048), cos/sin (2048,64), 16 heads, head_dim 128.
"""

import sys

sys.path.insert(0, "/opt/trn_rl_repo")

import numpy as np

import concourse.bass as bass
import concourse.tile as tile
from concourse import bacc, mybir
from concourse.bass_utils import run_bass_kernel_spmd

B, T, D, H = 2, 2048, 2048, 16
HD, HALF = 128, 64
TPC = 4          # heads per core
NT = T // 128    # 16 t-tiles
NK = D // 128    # 16 contraction chunks for the projections
NG = T // 512    # 4 q-groups per head
SCALE = float(1.0 / np.sqrt(HD))
FP32 = mybir.dt.float32
MM_DT = mybir.dt.float32r
EXP = mybir.ActivationFunctionType.Exp


def build_program():
    nc = bacc.Bacc("TRN2", target_bir_lowering=False, debug=False)

    xT = nc.dram_tensor("xT", [D, T], MM_DT, kind="ExternalInput").ap()
    wqk = nc.dram_tensor("wqk", [D, 2 * TPC * HD], MM_DT, kind="ExternalInput").ap()
    wv = nc.dram_tensor("wv", [D, TPC * HD], MM_DT, kind="ExternalInput").ap()
    wp = nc.dram_tensor("wp", [TPC * HD, D], MM_DT, kind="ExternalInput").ap()
    cos = nc.dram_tensor("cos", [T, HALF], FP32, kind="ExternalInput").ap()
    sin = nc.dram_tensor("sin", [T, HALF], FP32, kind="ExternalInput").ap()
    maskl = nc.dram_tensor("maskl", [128, 128], FP32, kind="ExternalInput").ap()
    ident = nc.dram_tensor("ident", [128, 128], MM_DT, kind="ExternalInput").ap()
    ones = nc.dram_tensor("ones", [128, 1], MM_DT, kind="ExternalInput").ap()
    outT = nc.dram_tensor("outT", [D, T], FP32, kind="ExternalOutput").ap()

    with tile.TileContext(nc) as tc:
        _kernel(tc, xT, wqk, wv, wp, cos, sin, maskl, ident, ones, outT)
    nc.compile()
    return nc


def _kernel(tc, xT, wqk, wv, wp, cos, sin, maskl, ident, ones, outT):
    nc = tc.nc
    NQK = 2 * TPC * HD  # 1024 qk output columns
    NV = TPC * HD       # 512 v output columns

    from contextlib import ExitStack

    with ExitStack() as top:
        # ---- persistent pools ----
        consts = top.enter_context(tc.tile_pool(name="consts", bufs=1))
        qt_pool = top.enter_context(tc.tile_pool(name="qt", bufs=TPC))
        kt_pool = top.enter_context(tc.tile_pool(name="kt", bufs=TPC))
        # PSUM: psS 4 banks + psO 2 + psR 2 = 8
        psS = top.enter_context(tc.tile_pool(name="psS", bufs=3, space="PSUM"))
        psO = top.enter_context(tc.tile_pool(name="psO", bufs=3, space="PSUM"))
        psR = top.enter_context(tc.tile_pool(name="psR", bufs=2, space="PSUM"))

        l_tile = consts.tile([128, 128], FP32)
        nc.sync.dma_start(out=l_tile, in_=maskl)
        id_tile = consts.tile([128, 128], MM_DT)
        nc.sync.dma_start(out=id_tile, in_=ident)
        ones_t = consts.tile([128, 1], MM_DT)
        nc.sync.dma_start(out=ones_t, in_=ones)

        # QT/KT: per head, (128 hd, T)
        QT = [qt_pool.tile([128, T], MM_DT, tag="qt", name=f"QT{i}") for i in range(TPC)]
        KT = [kt_pool.tile([128, T], MM_DT, tag="kt", name=f"KT{i}") for i in range(TPC)]

        # ================= phase 1a: qk projection + rope + transpose ======
        with tc.tile_pool(name="wqk_cache", bufs=NK) as wqk_pool, \
             tc.tile_pool(name="x_stream", bufs=32) as x_pool, \
             tc.tile_pool(name="qk_evict", bufs=4) as qk_pool, \
             tc.tile_pool(name="rope_tmp", bufs=8) as rope_pool, \
             tc.tile_pool(name="cs", bufs=4) as cs_pool:

            # cache all of wqk in SBUF (8MB), reused by all 16 t-tiles
            WQK = []
            for k in range(NK):
                w = wqk_pool.tile([128, NQK], MM_DT, tag="wqk")
                nc.sync.dma_start(out=w, in_=wqk[k * 128 : (k + 1) * 128, :])
                WQK.append(w)

            for t in range(NT):
                psQ = psS.tile([128, 512], FP32, tag="psS")
                psK = psS.tile([128, 512], FP32, tag="psS")
                for k in range(NK):
                    xt = x_pool.tile([128, 128], MM_DT, tag="x")
                    nc.sync.dma_start(
                        out=xt, in_=xT[k * 128 : (k + 1) * 128, t * 128 : (t + 1) * 128]
                    )
                    nc.tensor.matmul(psQ, xt, WQK[k][:, 0:512],
                                     start=(k == 0), stop=(k == NK - 1))
                    nc.tensor.matmul(psK, xt, WQK[k][:, 512:1024],
                                     start=(k == 0), stop=(k == NK - 1))
                ct = cs_pool.tile([128, HALF], FP32, tag="c")
                nc.sync.dma_start(out=ct, in_=cos[t * 128 : (t + 1) * 128, :])
                st = cs_pool.tile([128, HALF], FP32, tag="s")
                nc.sync.dma_start(out=st, in_=sin[t * 128 : (t + 1) * 128, :])
                # broadcast (128, 64) -> (128, 2, 64) with 0-step middle dim
                c_b = ct.unsqueeze(1).broadcast_to((128, 2, HALF))
                s_b = st.unsqueeze(1).broadcast_to((128, 2, HALF))

                for hh in range(TPC):
                    for which in range(2):  # 0 = Q, 1 = K
                        # rope reads the projection psum directly (DVE can
                        # read PSUM); no SBUF eviction hop needed
                        srcp = psQ if which == 0 else psK
                        blk = srcp[:, hh * HD : (hh + 1) * HD]
                        pair = blk.rearrange("p (two h) -> p two h", two=2)
                        t_a = rope_pool.tile([128, 2, HALF], FP32, tag="ta")
                        t_b = rope_pool.tile([128, 2, HALF], FP32, tag="tb")
                        nc.vector.tensor_mul(t_a, pair, c_b)
                        nc.vector.tensor_mul(t_b, pair, s_b)
                        ro = rope_pool.tile([128, HALF, 2], MM_DT, tag="ro")
                        # out[:, i, 0] = x1*c - x2*s ; out[:, i, 1] = x1*s + x2*c
                        nc.vector.tensor_sub(ro[:, :, 0:1],
                                             t_a[:, 0, :].unsqueeze(2),
                                             t_b[:, 1, :].unsqueeze(2))
                        nc.vector.tensor_add(ro[:, :, 1:2],
                                             t_b[:, 0, :].unsqueeze(2),
                                             t_a[:, 1, :].unsqueeze(2))
                        ro_flat = ro.rearrange("p h two -> p (h two)")
                        # transpose (128t, 128hd) -> (128hd, 128t)
                        pst = psO.tile([128, 512], MM_DT, tag="psO")
                        nc.tensor.transpose(pst[:, :128], ro_flat, id_tile)
                        dst = QT[hh] if which == 0 else KT[hh]
                        # evict on ACT: DVE is the loaded engine in this phase
                        nc.scalar.copy(
                            out=dst[:, t * 128 : (t + 1) * 128], in_=pst[:, :128]
                        )

        # ========== phase 2: v projection interleaved with attention ======
        # V: per t-tile, (128 t, 512) with 4 head column groups. The V
        # projection for q-group g's new t-tiles is emitted just before
        # group g's attention so its matmuls fill PE slack left by the
        # exp-paced attention pipeline (and its xT DMAs spread out).
        v_pool = top.enter_context(tc.tile_pool(name="v", bufs=NT))
        V = [v_pool.tile([128, NV], MM_DT, tag="v", name=f"V{i}") for i in range(NT)]
        # attention output transposed: per head, (128 hd, T)
        o_pool = top.enter_context(tc.tile_pool(name="o", bufs=TPC))
        OT = [o_pool.tile([128, T], MM_DT, tag="o", name=f"OT{i}") for i in range(TPC)]
        with tc.tile_pool(name="wv_cache", bufs=NK) as wv_pool, \
             tc.tile_pool(name="x_stream2", bufs=20) as x2_pool, \
             tc.tile_pool(name="p_sb", bufs=6) as p_pool, \
             tc.tile_pool(name="rs_sb", bufs=4) as rs_pool, \
             tc.tile_pool(name="rb_sb", bufs=3) as rb_pool:
            WV = []
            for k in range(NK):
                w = wv_pool.tile([128, NV], MM_DT, tag="wv")
                nc.sync.dma_start(out=w, in_=wv[k * 128 : (k + 1) * 128, :])
                WV.append(w)
            for g in range(NG):
                for t in range(4 * g, 4 * g + 4):
                    ps = psS.tile([128, 512], FP32, tag="psS")
                    for k in range(NK):
                        xt = x2_pool.tile([128, 128], MM_DT, tag="x2")
                        nc.sync.dma_start(
                            out=xt,
                            in_=xT[k * 128 : (k + 1) * 128, t * 128 : (t + 1) * 128],
                        )
                        nc.tensor.matmul(ps, xt, WV[k],
                                         start=(k == 0), stop=(k == NK - 1))
                    nc.vector.tensor_copy(out=V[t], in_=ps)
                for hh in range(TPC):
                    qcol0 = g * 512
                    nchunks = 4 * g + 4
                    po = psO.tile([128, 512], FP32, tag="psO")
                    rs = psR.tile([1, 512], FP32, tag="psR")
                    for kj in range(nchunks):
                        s0 = max(0, kj - 4 * g)   # first unmasked 128-q sub
                        off = s0 * 128
                        w = 512 - off
                        ps = psS.tile([128, 512], FP32, tag="psS")
                        # scoresT chunk (128 k, w q)
                        nc.tensor.matmul(
                            ps[:, :w],
                            KT[hh][:, kj * 128 : (kj + 1) * 128],
                            QT[hh][:, qcol0 + off : qcol0 + 512],
                            start=True, stop=True,
                        )
                        sd = kj - 4 * g
                        if 0 <= sd <= 3:
                            dcol = sd * 128 - off
                            nc.vector.tensor_sub(
                                ps[:, dcol : dcol + 128],
                                ps[:, dcol : dcol + 128],
                                l_tile,
                            )
                        pt = p_pool.tile([128, 512], MM_DT, tag="p")
                        nc.scalar.activation(out=pt[:, :w], in_=ps[:, :w],
                                             func=EXP, scale=SCALE)
                        # rowsum over k (ones-vector matmul), psum-accumulated
                        nc.tensor.matmul(rs[:, off:512], ones_t, pt[:, :w],
                                         start=(kj == 0), stop=(kj == nchunks - 1))
                        # PV accumulate: (128 hd, w q)
                        nc.tensor.matmul(po[:, off:512],
                                         V[kj][:, hh * HD : (hh + 1) * HD],
                                         pt[:, :w],
                                         start=(kj == 0), stop=(kj == nchunks - 1))
                    rr = rs_pool.tile([1, 512], FP32, tag="rr")
                    nc.vector.tensor_copy(out=rr, in_=rs)
                    rrep = rb_pool.tile([128, 512], FP32, tag="rrep")
                    nc.gpsimd.partition_broadcast(rrep, rr)
                    nc.vector.reciprocal(rrep, rrep)
                    nc.vector.tensor_mul(OT[hh][:, qcol0 : qcol0 + 512], po, rrep)

        # ================= phase 3: output projection =====================
        with tc.tile_pool(name="wp_stream", bufs=8) as wp_pool, \
             tc.tile_pool(name="out_evict", bufs=4) as out_pool:
            for m in range(NK):  # 16 blocks of 128 output (D) rows
                WPm = []
                for hh in range(TPC):
                    wt = wp_pool.tile([128, 128], MM_DT, tag="wp", name=f"wt{m}_{hh}")
                    nc.sync.dma_start(
                        out=wt,
                        in_=wp[hh * 128 : (hh + 1) * 128, m * 128 : (m + 1) * 128],
                    )
                    WPm.append(wt)
                for c4 in range(4):  # 512-wide T chunks
                    ps = psO.tile([128, 512], FP32, tag="psO")
                    for hh in range(TPC):
                        nc.tensor.matmul(
                            ps,
                            WPm[hh],
                            OT[hh][:, c4 * 512 : (c4 + 1) * 512],
                            start=(hh == 0), stop=(hh == TPC - 1),
                        )
                    ob = out_pool.tile([128, 512], FP32, tag="ob")
                    nc.scalar.copy(out=ob, in_=ps)
                    nc.sync.dma_start(
                        out=outT[m * 128 : (m + 1) * 128, c4 * 512 : (c4 + 1) * 512],
                        in_=ob,
                    )




_PROGRAM = None


def _get_program():
    global _PROGRAM
    if _PROGRAM is None:
        _PROGRAM = build_program()
    return _PROGRAM


def _make_in_maps(x, cos, sin, Wqkv, Wproj):
    maskl = (np.tril(np.ones((128, 128), np.float32), -1) * 1e30).astype(np.float32)
    ident = np.eye(128, dtype=np.float32)
    ones = np.ones((128, 1), dtype=np.float32)
    in_maps = []
    for c in range(8):
        b, hg = c // 4, c % 4
        h0 = hg * TPC
        in_maps.append({
            "xT": np.ascontiguousarray(x[b].T),
            "wqk": np.ascontiguousarray(np.concatenate(
                [Wqkv[:, h0 * HD : (h0 + TPC) * HD],
                 Wqkv[:, D + h0 * HD : D + (h0 + TPC) * HD]], axis=1)),
            "wv": np.ascontiguousarray(Wqkv[:, 2 * D + h0 * HD : 2 * D + (h0 + TPC) * HD]),
            "wp": np.ascontiguousarray(Wproj[h0 * HD : (h0 + TPC) * HD, :]),
            "cos": np.asarray(cos, np.float32),
            "sin": np.asarray(sin, np.float32),
            "maskl": maskl,
            "ident": ident,
            "ones": ones,
        })
    return in_maps


def _combine(results):
    outs = []
    for b in range(2):
        acc = results[4 * b]["outT"].astype(np.float32)
        for hg in range(1, 4):
            acc = acc + results[4 * b + hg]["outT"]
        outs.append(acc.T)
    return np.ascontiguousarray(np.stack(outs))


def kernel(x, cos, sin, Wqkv, Wproj):
    nc = _get_program()
    in_maps = _make_in_maps(np.asarray(x, np.float32), cos, sin,
                            np.asarray(Wqkv, np.float32), np.asarray(Wproj, np.float32))
    res = run_bass_kernel_spmd(nc, in_maps, list(range(8)))
    return _combine(res.results)


def _install_ntff_shim():
    """Provide the antenv.axon_hooks registry this container lacks, wired to
    the ctypes NTFF hook from trn_agent_boot, so trace=True works."""
    import types

    if "antenv.axon_hooks" in sys.modules:
        return
    hook = None
    try:
        from trn_agent_boot.trn_boot import _ntff_profile_via_ctypes
        hook = _ntff_profile_via_ctypes("/opt/axon/libaxon_pjrt.so")
    except Exception as e:
        print("ntff shim unavailable:", e)
    mod = types.ModuleType("antenv.axon_hooks")
    mod._hook = hook
    mod.get_axon_ntff_profile_hook = lambda: mod._hook
    mod.set_axon_ntff_profile_hook = lambda h: setattr(mod, "_hook", h)
    sys.modules["antenv.axon_hooks"] = mod
    # keep artifacts local; the bucket upload path isn't available here
    import concourse.bass_utils as bu
    bu.upload_artifacts = lambda tmpdir: tmpdir


def kernel_profiled(x, cos, sin, Wqkv, Wproj, trace_cores=None, tmpdir=None):
    nc = _get_program()
    _install_ntff_shim()
    in_maps = _make_in_maps(np.asarray(x, np.float32), cos, sin,
                            np.asarray(Wqkv, np.float32), np.asarray(Wproj, np.float32))
    res = run_bass_kernel_spmd(nc, in_maps, list(range(8)), trace=True,
                               trace_cores=trace_cores, tmpdir=tmpdir)
    return _combine(res.results), res



# revision 2
# speedup vs baseline: 1.3492x; 1.3492x over previous
"""Causal self-attention with RoPE on 8 Trainium2 NeuronCores.

Sharding: tensor-parallel over heads (4 groups of 4 heads) x data-parallel
over batch (2), one (batch, head-group) pair per core. Each core computes
its heads' QKV projection, RoPE, causal attention, and a row-slice of the
output projection; the host sums the 4 partial projections per batch.

All matmul operands are bf16 (1 cycle/col on the PE vs ~1.8 measured for
float32r); accumulation stays fp32 in PSUM. The QKV and V projections are
fused into a single pass over x (x is DMA'd once), attention scores are
software-pipelined one chunk ahead of the exp/rowsum/PV consumers so the
PE never waits on the Act engine, and Wproj is prefetched at kernel start.

Attention computes scores transposed (k on partitions, q on the free dim,
512-wide q-groups): softmax rowsums come from a ones-vector matmul, the
probabilities feed P@V directly as the moving operand, and no per-block
transposes of the probability matrix are needed.

Hardcoded problem shape: x (2,2048,2048), Wqkv (2048,6144), Wproj
(2048,2048), cos/sin (2048,64), 16 heads, head_dim 128.
"""

import sys

sys.path.insert(0, "/opt/trn_rl_repo")

import numpy as np
import ml_dtypes

import concourse.bass as bass
import concourse.tile as tile
from concourse import bacc, mybir
from concourse.bass_utils import run_bass_kernel_spmd

B, T, D, H = 2, 2048, 2048, 16
HD, HALF = 128, 64
TPC = 4          # heads per core
NT = T // 128    # 16 t-tiles
NK = D // 128    # 16 contraction chunks for the projections
NG = T // 512    # 4 q-groups per head
SCALE = float(1.0 / np.sqrt(HD))
FP32 = mybir.dt.float32
MM_DT = mybir.dt.bfloat16
NP_MM = ml_dtypes.bfloat16
EXP = mybir.ActivationFunctionType.Exp


def build_program():
    nc = bacc.Bacc("TRN2", target_bir_lowering=False, debug=False)

    xT = nc.dram_tensor("xT", [D, T], MM_DT, kind="ExternalInput").ap()
    wqk = nc.dram_tensor("wqk", [D, 2 * TPC * HD], MM_DT, kind="ExternalInput").ap()
    wv = nc.dram_tensor("wv", [D, TPC * HD], MM_DT, kind="ExternalInput").ap()
    wp = nc.dram_tensor("wp", [TPC * HD, D], MM_DT, kind="ExternalInput").ap()
    cos = nc.dram_tensor("cos", [T, HALF], FP32, kind="ExternalInput").ap()
    sin = nc.dram_tensor("sin", [T, HALF], FP32, kind="ExternalInput").ap()
    maskl = nc.dram_tensor("maskl", [128, 128], FP32, kind="ExternalInput").ap()
    ident = nc.dram_tensor("ident", [128, 128], MM_DT, kind="ExternalInput").ap()
    ones = nc.dram_tensor("ones", [128, 1], MM_DT, kind="ExternalInput").ap()
    outT = nc.dram_tensor("outT", [D, T], MM_DT, kind="ExternalOutput").ap()

    with tile.TileContext(nc) as tc:
        _kernel(tc, xT, wqk, wv, wp, cos, sin, maskl, ident, ones, outT)
    nc.compile()
    return nc


def _kernel(tc, xT, wqk, wv, wp, cos, sin, maskl, ident, ones, outT):
    nc = tc.nc
    NQK = 2 * TPC * HD  # 1024 qk output columns
    NV = TPC * HD       # 512 v output columns

    from contextlib import ExitStack

    with ExitStack() as top:
        # ---- persistent pools ----
        consts = top.enter_context(tc.tile_pool(name="consts", bufs=1))
        wp_pool = top.enter_context(tc.tile_pool(name="wp", bufs=TPC))
        qt_pool = top.enter_context(tc.tile_pool(name="qt", bufs=TPC))
        kt_pool = top.enter_context(tc.tile_pool(name="kt", bufs=TPC))
        v_pool = top.enter_context(tc.tile_pool(name="v", bufs=NT))
        o_pool = top.enter_context(tc.tile_pool(name="o", bufs=TPC))

        l_tile = consts.tile([128, 128], FP32)
        nc.sync.dma_start(out=l_tile, in_=maskl)
        id_tile = consts.tile([128, 128], MM_DT)
        nc.sync.dma_start(out=id_tile, in_=ident)
        ones_t = consts.tile([128, 1], MM_DT)
        nc.sync.dma_start(out=ones_t, in_=ones)

        # prefetch all of Wproj (phase 3 never touches DRAM for weights)
        WP = []
        for hh in range(TPC):
            w = wp_pool.tile([128, D], MM_DT, tag="wp", name=f"WP{hh}")
            nc.sync.dma_start(out=w, in_=wp[hh * 128 : (hh + 1) * 128, :])
            WP.append(w)

        # QT/KT: per head, (128 hd, T); V: per t-tile (128 t, 4*HD)
        QT = [qt_pool.tile([128, T], MM_DT, tag="qt", name=f"QT{i}") for i in range(TPC)]
        KT = [kt_pool.tile([128, T], MM_DT, tag="kt", name=f"KT{i}") for i in range(TPC)]
        V = [v_pool.tile([128, NV], MM_DT, tag="v", name=f"V{i}") for i in range(NT)]
        # attention output transposed: per head, (128 hd, T)
        OT = [o_pool.tile([128, T], MM_DT, tag="o", name=f"OT{i}") for i in range(TPC)]

        # ===== phase 1: fused q/k/v projection + rope + transpose ==========
        # Single pass over x: per 128-row t-tile, accumulate Q, K and V
        # (psum) over the 16 D-chunks, then rope+transpose Q/K and evict V.
        with tc.tile_pool(name="wqk_cache", bufs=NK) as wqk_pool, \
             tc.tile_pool(name="wv_cache", bufs=NK) as wv_pool, \
             tc.tile_pool(name="x_stream", bufs=32) as x_pool, \
             tc.tile_pool(name="rope_tmp", bufs=8) as rope_pool, \
             tc.tile_pool(name="cs", bufs=4) as cs_pool, \
             tc.tile_pool(name="psA", bufs=6, space="PSUM") as psA, \
             tc.tile_pool(name="psT", bufs=2, space="PSUM") as psT:

            # cache all projection weights in SBUF, reused by all 16 t-tiles
            WQK = []
            WV = []
            for k in range(NK):
                w = wqk_pool.tile([128, NQK], MM_DT, tag="wqk")
                nc.sync.dma_start(out=w, in_=wqk[k * 128 : (k + 1) * 128, :])
                WQK.append(w)
                w = wv_pool.tile([128, NV], MM_DT, tag="wv")
                nc.sync.dma_start(out=w, in_=wv[k * 128 : (k + 1) * 128, :])
                WV.append(w)

            for t in range(NT):
                psQ = psA.tile([128, 512], FP32, tag="psA")
                psK = psA.tile([128, 512], FP32, tag="psA")
                psV = psA.tile([128, 512], FP32, tag="psA")
                for k in range(NK):
                    xt = x_pool.tile([128, 128], MM_DT, tag="x")
                    nc.sync.dma_start(
                        out=xt, in_=xT[k * 128 : (k + 1) * 128, t * 128 : (t + 1) * 128]
                    )
                    nc.tensor.matmul(psQ, xt, WQK[k][:, 0:512],
                                     start=(k == 0), stop=(k == NK - 1))
                    nc.tensor.matmul(psK, xt, WQK[k][:, 512:1024],
                                     start=(k == 0), stop=(k == NK - 1))
                    nc.tensor.matmul(psV, xt, WV[k],
                                     start=(k == 0), stop=(k == NK - 1))
                nc.vector.tensor_copy(out=V[t], in_=psV)
                ct = cs_pool.tile([128, HALF], FP32, tag="c")
                nc.sync.dma_start(out=ct, in_=cos[t * 128 : (t + 1) * 128, :])
                st = cs_pool.tile([128, HALF], FP32, tag="s")
                nc.sync.dma_start(out=st, in_=sin[t * 128 : (t + 1) * 128, :])
                # broadcast (128, 64) -> (128, 2, 64) with 0-step middle dim
                c_b = ct.unsqueeze(1).broadcast_to((128, 2, HALF))
                s_b = st.unsqueeze(1).broadcast_to((128, 2, HALF))

                for hh in range(TPC):
                    for which in range(2):  # 0 = Q, 1 = K
                        # rope reads the projection psum directly (DVE can
                        # read PSUM); no SBUF eviction hop needed
                        srcp = psQ if which == 0 else psK
                        blk = srcp[:, hh * HD : (hh + 1) * HD]
                        pair = blk.rearrange("p (two h) -> p two h", two=2)
                        t_a = rope_pool.tile([128, 2, HALF], FP32, tag="ta")
                        t_b = rope_pool.tile([128, 2, HALF], FP32, tag="tb")
                        nc.vector.tensor_mul(t_a, pair, c_b)
                        nc.vector.tensor_mul(t_b, pair, s_b)
                        ro = rope_pool.tile([128, HALF, 2], MM_DT, tag="ro")
                        # out[:, i, 0] = x1*c - x2*s ; out[:, i, 1] = x1*s + x2*c
                        nc.vector.tensor_sub(ro[:, :, 0:1],
                                             t_a[:, 0, :].unsqueeze(2),
                                             t_b[:, 1, :].unsqueeze(2))
                        nc.vector.tensor_add(ro[:, :, 1:2],
                                             t_b[:, 0, :].unsqueeze(2),
                                             t_a[:, 1, :].unsqueeze(2))
                        ro_flat = ro.rearrange("p h two -> p (h two)")
                        # transpose (128t, 128hd) -> (128hd, 128t)
                        pst = psT.tile([128, 512], MM_DT, tag="psT")
                        nc.tensor.transpose(pst[:, :128], ro_flat, id_tile)
                        dst = QT[hh] if which == 0 else KT[hh]
                        # evict on ACT: DVE is the loaded engine in this phase
                        nc.scalar.copy(
                            out=dst[:, t * 128 : (t + 1) * 128], in_=pst[:, :128]
                        )

        # ===== phase 2: causal attention, scores pipelined ahead ===========
        with tc.tile_pool(name="p_sb", bufs=6) as p_pool, \
             tc.tile_pool(name="rs_sb", bufs=4) as rs_pool, \
             tc.tile_pool(name="rb_sb", bufs=3) as rb_pool, \
             tc.tile_pool(name="psS", bufs=3, space="PSUM") as psS, \
             tc.tile_pool(name="psO", bufs=2, space="PSUM") as psO, \
             tc.tile_pool(name="psR", bufs=2, space="PSUM") as psR:
            for g in range(NG):
                qcol0 = g * 512
                nchunks = 4 * g + 4
                for hh in range(TPC):
                    po = psO.tile([128, 512], FP32, tag="psO")
                    rs = psR.tile([1, 512], FP32, tag="psR")
                    ps_chunks = {}

                    def emit_scores(kj):
                        s0 = max(0, kj - 4 * g)   # first unmasked 128-q sub
                        off = s0 * 128
                        w = 512 - off
                        ps = psS.tile([128, 512], FP32, tag="psS")
                        # scoresT chunk (128 k, w q)
                        nc.tensor.matmul(
                            ps[:, :w],
                            KT[hh][:, kj * 128 : (kj + 1) * 128],
                            QT[hh][:, qcol0 + off : qcol0 + 512],
                            start=True, stop=True,
                        )
                        ps_chunks[kj] = ps

                    emit_scores(0)
                    for kj in range(nchunks):
                        if kj + 1 < nchunks:
                            emit_scores(kj + 1)
                        s0 = max(0, kj - 4 * g)
                        off = s0 * 128
                        w = 512 - off
                        ps = ps_chunks.pop(kj)
                        sd = kj - 4 * g
                        if 0 <= sd <= 3:
                            dcol = sd * 128 - off
                            nc.vector.tensor_sub(
                                ps[:, dcol : dcol + 128],
                                ps[:, dcol : dcol + 128],
                                l_tile,
                            )
                        pt = p_pool.tile([128, 512], MM_DT, tag="p")
                        nc.scalar.activation(out=pt[:, :w], in_=ps[:, :w],
                                             func=EXP, scale=SCALE)
                        # rowsum over k (ones-vector matmul), psum-accumulated
                        nc.tensor.matmul(rs[:, off:512], ones_t, pt[:, :w],
                                         start=(kj == 0), stop=(kj == nchunks - 1))
                        # PV accumulate: (128 hd, w q)
                        nc.tensor.matmul(po[:, off:512],
                                         V[kj][:, hh * HD : (hh + 1) * HD],
                                         pt[:, :w],
                                         start=(kj == 0), stop=(kj == nchunks - 1))
                    rr = rs_pool.tile([1, 512], FP32, tag="rr")
                    nc.vector.reciprocal(rr, rs)
                    rrep = rb_pool.tile([128, 512], FP32, tag="rrep")
                    nc.gpsimd.partition_broadcast(rrep, rr)
                    nc.vector.tensor_mul(OT[hh][:, qcol0 : qcol0 + 512], po, rrep)

        # ================= phase 3: output projection =====================
        with tc.tile_pool(name="out_evict", bufs=4) as out_pool, \
             tc.tile_pool(name="psC", bufs=3, space="PSUM") as psC:
            for m in range(NK):  # 16 blocks of 128 output (D) rows
                for c4 in range(4):  # 512-wide T chunks
                    ps = psC.tile([128, 512], FP32, tag="psC")
                    for hh in range(TPC):
                        nc.tensor.matmul(
                            ps,
                            WP[hh][:, m * 128 : (m + 1) * 128],
                            OT[hh][:, c4 * 512 : (c4 + 1) * 512],
                            start=(hh == 0), stop=(hh == TPC - 1),
                        )
                    ob = out_pool.tile([128, 512], MM_DT, tag="ob")
                    nc.scalar.copy(out=ob, in_=ps)
                    nc.sync.dma_start(
                        out=outT[m * 128 : (m + 1) * 128, c4 * 512 : (c4 + 1) * 512],
                        in_=ob,
                    )


_PROGRAM = None


def _get_program():
    global _PROGRAM
    if _PROGRAM is None:
        _PROGRAM = build_program()
    return _PROGRAM


def _make_in_maps(x, cos, sin, Wqkv, Wproj):
    maskl = (np.tril(np.ones((128, 128), np.float32), -1) * 1e30).astype(np.float32)
    ident = np.eye(128, dtype=np.float32).astype(NP_MM)
    ones = np.ones((128, 1), dtype=np.float32).astype(NP_MM)
    in_maps = []
    for c in range(8):
        b, hg = c // 4, c % 4
        h0 = hg * TPC
        in_maps.append({
            "xT": np.ascontiguousarray(x[b].T).astype(NP_MM),
            "wqk": np.ascontiguousarray(np.concatenate(
                [Wqkv[:, h0 * HD : (h0 + TPC) * HD],
                 Wqkv[:, D + h0 * HD : D + (h0 + TPC) * HD]], axis=1)).astype(NP_MM),
            "wv": np.ascontiguousarray(
                Wqkv[:, 2 * D + h0 * HD : 2 * D + (h0 + TPC) * HD]).astype(NP_MM),
            "wp": np.ascontiguousarray(Wproj[h0 * HD : (h0 + TPC) * HD, :]).astype(NP_MM),
            "cos": np.asarray(cos, np.float32),
            "sin": np.asarray(sin, np.float32),
            "maskl": maskl,
            "ident": ident,
            "ones": ones,
        })
    return in_maps


def _combine(results):
    outs = []
    for b in range(2):
        acc = results[4 * b]["outT"].astype(np.float32)
        for hg in range(1, 4):
            acc = acc + results[4 * b + hg]["outT"].astype(np.float32)
        outs.append(acc.T)
    return np.ascontiguousarray(np.stack(outs))


def kernel(x, cos, sin, Wqkv, Wproj):
    nc = _get_program()
    in_maps = _make_in_maps(np.asarray(x, np.float32), cos, sin,
                            np.asarray(Wqkv, np.float32), np.asarray(Wproj, np.float32))
    res = run_bass_kernel_spmd(nc, in_maps, list(range(8)))
    return _combine(res.results)


def _install_ntff_shim():
    """Provide the antenv.axon_hooks registry this container lacks, wired to
    the ctypes NTFF hook from trn_agent_boot, so trace=True works."""
    import types

    if "antenv.axon_hooks" in sys.modules:
        return
    hook = None
    try:
        from trn_agent_boot.trn_boot import _ntff_profile_via_ctypes
        hook = _ntff_profile_via_ctypes("/opt/axon/libaxon_pjrt.so")
    except Exception as e:
        print("ntff shim unavailable:", e)
    mod = types.ModuleType("antenv.axon_hooks")
    mod._hook = hook
    mod.get_axon_ntff_profile_hook = lambda: mod._hook
    mod.set_axon_ntff_profile_hook = lambda h: setattr(mod, "_hook", h)
    sys.modules["antenv.axon_hooks"] = mod
    # keep artifacts local; the bucket upload path isn't available here
    import concourse.bass_utils as bu
    bu.upload_artifacts = lambda tmpdir: tmpdir


def kernel_profiled(x, cos, sin, Wqkv, Wproj, trace_cores=None, tmpdir=None):
    nc = _get_program()
    _install_ntff_shim()
    in_maps = _make_in_maps(np.asarray(x, np.float32), cos, sin,
                            np.asarray(Wqkv, np.float32), np.asarray(Wproj, np.float32))
    res = run_bass_kernel_spmd(nc, in_maps, list(range(8)), trace=True,
                               trace_cores=trace_cores, tmpdir=tmpdir)
    return _combine(res.results), res


# revision 5
# speedup vs baseline: 1.3641x; 1.0110x over previous
"""Causal self-attention with RoPE on 8 Trainium2 NeuronCores.

Sharding: tensor-parallel over heads (4 groups of 4 heads) x data-parallel
over batch (2), one (batch, head-group) pair per core. Each core computes
its heads' QKV projection, RoPE, causal attention, and a row-slice of the
output projection; the host sums the 4 partial projections per batch.

All matmul operands are bf16 (1 cycle/col on the PE vs ~1.8 measured for
float32r); accumulation stays fp32 in PSUM. The QKV and V projections are
fused into a single pass over x (x is DMA'd once), attention scores are
software-pipelined one chunk ahead of the exp/rowsum/PV consumers so the
PE never waits on the Act engine, and Wproj is prefetched at kernel start.

Attention computes scores transposed (k on partitions, q on the free dim,
512-wide q-groups): softmax rowsums come from a ones-vector matmul, the
probabilities feed P@V directly as the moving operand, and no per-block
transposes of the probability matrix are needed.

Hardcoded problem shape: x (2,2048,2048), Wqkv (2048,6144), Wproj
(2048,2048), cos/sin (2048,64), 16 heads, head_dim 128.
"""

import sys

sys.path.insert(0, "/opt/trn_rl_repo")

import numpy as np
import ml_dtypes

import concourse.bass as bass
import concourse.tile as tile
from concourse import bacc, mybir
from concourse.bass_utils import run_bass_kernel_spmd

B, T, D, H = 2, 2048, 2048, 16
HD, HALF = 128, 64
TPC = 4          # heads per core
NT = T // 128    # 16 t-tiles
NK = D // 128    # 16 contraction chunks for the projections
NG = T // 512    # 4 q-groups per head
SCALE = float(1.0 / np.sqrt(HD))
FP32 = mybir.dt.float32
MM_DT = mybir.dt.bfloat16
NP_MM = ml_dtypes.bfloat16
EXP = mybir.ActivationFunctionType.Exp


def build_program():
    nc = bacc.Bacc("TRN2", target_bir_lowering=False, debug=False)

    xT = nc.dram_tensor("xT", [D, T], MM_DT, kind="ExternalInput").ap()
    wqk = nc.dram_tensor("wqk", [D, 2 * TPC * HD], MM_DT, kind="ExternalInput").ap()
    wv = nc.dram_tensor("wv", [D, TPC * HD], MM_DT, kind="ExternalInput").ap()
    wp = nc.dram_tensor("wp", [TPC * HD, D], MM_DT, kind="ExternalInput").ap()
    cos = nc.dram_tensor("cos", [T, HALF], FP32, kind="ExternalInput").ap()
    sin = nc.dram_tensor("sin", [T, HALF], FP32, kind="ExternalInput").ap()
    maskl = nc.dram_tensor("maskl", [128, 128], FP32, kind="ExternalInput").ap()
    ident = nc.dram_tensor("ident", [128, 128], MM_DT, kind="ExternalInput").ap()
    ones = nc.dram_tensor("ones", [128, 1], MM_DT, kind="ExternalInput").ap()
    outT = nc.dram_tensor("outT", [D, T], MM_DT, kind="ExternalOutput").ap()

    with tile.TileContext(nc) as tc:
        _kernel(tc, xT, wqk, wv, wp, cos, sin, maskl, ident, ones, outT)
    nc.compile()
    return nc


def _kernel(tc, xT, wqk, wv, wp, cos, sin, maskl, ident, ones, outT):
    nc = tc.nc
    NQK = 2 * TPC * HD  # 1024 qk output columns
    NV = TPC * HD       # 512 v output columns

    from contextlib import ExitStack

    with ExitStack() as top:
        # ---- persistent pools ----
        consts = top.enter_context(tc.tile_pool(name="consts", bufs=1))
        wp_pool = top.enter_context(tc.tile_pool(name="wp", bufs=TPC))
        qt_pool = top.enter_context(tc.tile_pool(name="qt", bufs=TPC))
        kt_pool = top.enter_context(tc.tile_pool(name="kt", bufs=TPC))
        v_pool = top.enter_context(tc.tile_pool(name="v", bufs=NT))
        o_pool = top.enter_context(tc.tile_pool(name="o", bufs=TPC))

        l_tile = consts.tile([128, 128], FP32)
        nc.sync.dma_start(out=l_tile, in_=maskl)
        id_tile = consts.tile([128, 128], MM_DT)
        nc.sync.dma_start(out=id_tile, in_=ident)
        ones_t = consts.tile([128, 1], MM_DT)
        nc.sync.dma_start(out=ones_t, in_=ones)

        # prefetch all of Wproj (phase 3 never touches DRAM for weights)
        WP = []
        for hh in range(TPC):
            w = wp_pool.tile([128, D], MM_DT, tag="wp", name=f"WP{hh}")
            nc.sync.dma_start(out=w, in_=wp[hh * 128 : (hh + 1) * 128, :])
            WP.append(w)

        # QT/KT: per head, (128 hd, T); V: per t-tile (128 t, 4*HD)
        QT = [qt_pool.tile([128, T], MM_DT, tag="qt", name=f"QT{i}") for i in range(TPC)]
        KT = [kt_pool.tile([128, T], MM_DT, tag="kt", name=f"KT{i}") for i in range(TPC)]
        V = [v_pool.tile([128, NV], MM_DT, tag="v", name=f"V{i}") for i in range(NT)]
        # attention output transposed: per head, (128 hd, T)
        OT = [o_pool.tile([128, T], MM_DT, tag="o", name=f"OT{i}") for i in range(TPC)]

        # ===== phase 1: fused q/k/v projection + rope + transpose ==========
        # Single pass over x: per 128-row t-tile, accumulate Q, K and V
        # (psum) over the 16 D-chunks, then rope+transpose Q/K and evict V.
        with tc.tile_pool(name="wqk_cache", bufs=NK) as wqk_pool, \
             tc.tile_pool(name="wv_cache", bufs=NK) as wv_pool, \
             tc.tile_pool(name="x_stream", bufs=32) as x_pool, \
             tc.tile_pool(name="rope_tmp", bufs=4) as rope_pool, \
             tc.tile_pool(name="ro_sb", bufs=18) as ro_pool, \
             tc.tile_pool(name="cs", bufs=4) as cs_pool, \
             tc.tile_pool(name="psA", bufs=6, space="PSUM") as psA, \
             tc.tile_pool(name="psT", bufs=2, space="PSUM") as psT:

            # cache all projection weights in SBUF, reused by all 16 t-tiles
            WQK = []
            WV = []
            for k in range(NK):
                w = wqk_pool.tile([128, NQK], MM_DT, tag="wqk")
                nc.sync.dma_start(out=w, in_=wqk[k * 128 : (k + 1) * 128, :])
                WQK.append(w)
                w = wv_pool.tile([128, NV], MM_DT, tag="wv")
                nc.sync.dma_start(out=w, in_=wv[k * 128 : (k + 1) * 128, :])
                WV.append(w)

            def emit_transposes(pend):
                # transposes of tile t-1, emitted under tile t's matmuls so
                # the PE never waits on the serial DVE rope chain
                for ro_flat, dst, tcol in pend:
                    pst = psT.tile([128, 512], MM_DT, tag="psT")
                    nc.tensor.transpose(pst[:, :128], ro_flat, id_tile)
                    nc.scalar.copy(
                        out=dst[:, tcol : tcol + 128], in_=pst[:, :128]
                    )

            pend_tr = []
            for t in range(NT):
                psQ = psA.tile([128, 512], FP32, tag="psA")
                psK = psA.tile([128, 512], FP32, tag="psA")
                psV = psA.tile([128, 512], FP32, tag="psA")
                for k in range(NK):
                    xt = x_pool.tile([128, 128], MM_DT, tag="x")
                    nc.sync.dma_start(
                        out=xt, in_=xT[k * 128 : (k + 1) * 128, t * 128 : (t + 1) * 128]
                    )
                    nc.tensor.matmul(psQ, xt, WQK[k][:, 0:512],
                                     start=(k == 0), stop=(k == NK - 1))
                    nc.tensor.matmul(psK, xt, WQK[k][:, 512:1024],
                                     start=(k == 0), stop=(k == NK - 1))
                    nc.tensor.matmul(psV, xt, WV[k],
                                     start=(k == 0), stop=(k == NK - 1))
                    if k == 3:
                        # prior tile's rope outputs are ready by now; slot its
                        # transposes between this tile's projection matmuls
                        emit_transposes(pend_tr)
                        pend_tr = []
                # evict V on ACT: DVE is busy with rope in this phase
                nc.scalar.copy(out=V[t], in_=psV)
                ct = cs_pool.tile([128, HALF], FP32, tag="c")
                nc.sync.dma_start(out=ct, in_=cos[t * 128 : (t + 1) * 128, :])
                st = cs_pool.tile([128, HALF], FP32, tag="s")
                nc.sync.dma_start(out=st, in_=sin[t * 128 : (t + 1) * 128, :])
                # broadcast (128, 64) -> (128, 2, 64) with 0-step middle dim
                c_b = ct.unsqueeze(1).broadcast_to((128, 2, HALF))
                s_b = st.unsqueeze(1).broadcast_to((128, 2, HALF))

                for hh in range(TPC):
                    for which in range(2):  # 0 = Q, 1 = K
                        # rope reads the projection psum directly (DVE can
                        # read PSUM); no SBUF eviction hop needed
                        srcp = psQ if which == 0 else psK
                        blk = srcp[:, hh * HD : (hh + 1) * HD]
                        pair = blk.rearrange("p (two h) -> p two h", two=2)
                        t_a = rope_pool.tile([128, 2, HALF], FP32, tag="ta")
                        t_b = rope_pool.tile([128, 2, HALF], FP32, tag="tb")
                        nc.vector.tensor_mul(t_a, pair, c_b)
                        nc.vector.tensor_mul(t_b, pair, s_b)
                        ro = ro_pool.tile([128, HALF, 2], MM_DT, tag="ro")
                        # out[:, i, 0] = x1*c - x2*s ; out[:, i, 1] = x1*s + x2*c
                        nc.vector.tensor_sub(ro[:, :, 0:1],
                                             t_a[:, 0, :].unsqueeze(2),
                                             t_b[:, 1, :].unsqueeze(2))
                        nc.vector.tensor_add(ro[:, :, 1:2],
                                             t_b[:, 0, :].unsqueeze(2),
                                             t_a[:, 1, :].unsqueeze(2))
                        ro_flat = ro.rearrange("p h two -> p (h two)")
                        dst = QT[hh] if which == 0 else KT[hh]
                        pend_tr.append((ro_flat, dst, t * 128))
            emit_transposes(pend_tr)

        # ===== phase 2: causal attention, scores pipelined ahead ===========
        with tc.tile_pool(name="p_sb", bufs=6) as p_pool, \
             tc.tile_pool(name="rs_sb", bufs=4) as rs_pool, \
             tc.tile_pool(name="rb_sb", bufs=3) as rb_pool, \
             tc.tile_pool(name="psS", bufs=4, space="PSUM") as psS, \
             tc.tile_pool(name="psO", bufs=2, space="PSUM") as psO, \
             tc.tile_pool(name="psR", bufs=2, space="PSUM") as psR:
            for g in range(NG):
                qcol0 = g * 512
                nchunks = 4 * g + 4
                for hh in range(TPC):
                    po = psO.tile([128, 512], FP32, tag="psO")
                    rs = psR.tile([1, 512], FP32, tag="psR")
                    ps_chunks = {}

                    def emit_scores(kj):
                        s0 = max(0, kj - 4 * g)   # first unmasked 128-q sub
                        off = s0 * 128
                        w = 512 - off
                        ps = psS.tile([128, 512], FP32, tag="psS")
                        # scoresT chunk (128 k, w q)
                        nc.tensor.matmul(
                            ps[:, :w],
                            KT[hh][:, kj * 128 : (kj + 1) * 128],
                            QT[hh][:, qcol0 + off : qcol0 + 512],
                            start=True, stop=True,
                        )
                        ps_chunks[kj] = ps

                    emit_scores(0)
                    if nchunks > 1:
                        emit_scores(1)
                    for kj in range(nchunks):
                        if kj + 2 < nchunks:
                            emit_scores(kj + 2)
                        s0 = max(0, kj - 4 * g)
                        off = s0 * 128
                        w = 512 - off
                        ps = ps_chunks.pop(kj)
                        sd = kj - 4 * g
                        if 0 <= sd <= 3:
                            dcol = sd * 128 - off
                            nc.vector.tensor_sub(
                                ps[:, dcol : dcol + 128],
                                ps[:, dcol : dcol + 128],
                                l_tile,
                            )
                        pt = p_pool.tile([128, 512], MM_DT, tag="p")
                        nc.scalar.activation(out=pt[:, :w], in_=ps[:, :w],
                                             func=EXP, scale=SCALE)
                        # rowsum over k (ones-vector matmul), psum-accumulated
                        nc.tensor.matmul(rs[:, off:512], ones_t, pt[:, :w],
                                         start=(kj == 0), stop=(kj == nchunks - 1))
                        # PV accumulate: (128 hd, w q)
                        nc.tensor.matmul(po[:, off:512],
                                         V[kj][:, hh * HD : (hh + 1) * HD],
                                         pt[:, :w],
                                         start=(kj == 0), stop=(kj == nchunks - 1))
                    rr = rs_pool.tile([1, 512], FP32, tag="rr")
                    nc.vector.reciprocal(rr, rs)
                    rrep = rb_pool.tile([128, 512], FP32, tag="rrep")
                    nc.gpsimd.partition_broadcast(rrep, rr)
                    nc.vector.tensor_mul(OT[hh][:, qcol0 : qcol0 + 512], po, rrep)

        # ================= phase 3: output projection =====================
        with tc.tile_pool(name="out_evict", bufs=4) as out_pool, \
             tc.tile_pool(name="psC", bufs=3, space="PSUM") as psC:
            for m in range(NK):  # 16 blocks of 128 output (D) rows
                for c4 in range(4):  # 512-wide T chunks
                    ps = psC.tile([128, 512], FP32, tag="psC")
                    for hh in range(TPC):
                        nc.tensor.matmul(
                            ps,
                            WP[hh][:, m * 128 : (m + 1) * 128],
                            OT[hh][:, c4 * 512 : (c4 + 1) * 512],
                            start=(hh == 0), stop=(hh == TPC - 1),
                        )
                    ob = out_pool.tile([128, 512], MM_DT, tag="ob")
                    nc.scalar.copy(out=ob, in_=ps)
                    nc.sync.dma_start(
                        out=outT[m * 128 : (m + 1) * 128, c4 * 512 : (c4 + 1) * 512],
                        in_=ob,
                    )


_PROGRAM = None


def _get_program():
    global _PROGRAM
    if _PROGRAM is None:
        _PROGRAM = build_program()
    return _PROGRAM


def _make_in_maps(x, cos, sin, Wqkv, Wproj):
    maskl = (np.tril(np.ones((128, 128), np.float32), -1) * 1e30).astype(np.float32)
    ident = np.eye(128, dtype=np.float32).astype(NP_MM)
    ones = np.ones((128, 1), dtype=np.float32).astype(NP_MM)
    in_maps = []
    for c in range(8):
        b, hg = c // 4, c % 4
        h0 = hg * TPC
        in_maps.append({
            "xT": np.ascontiguousarray(x[b].T).astype(NP_MM),
            "wqk": np.ascontiguousarray(np.concatenate(
                [Wqkv[:, h0 * HD : (h0 + TPC) * HD],
                 Wqkv[:, D + h0 * HD : D + (h0 + TPC) * HD]], axis=1)).astype(NP_MM),
            "wv": np.ascontiguousarray(
                Wqkv[:, 2 * D + h0 * HD : 2 * D + (h0 + TPC) * HD]).astype(NP_MM),
            "wp": np.ascontiguousarray(Wproj[h0 * HD : (h0 + TPC) * HD, :]).astype(NP_MM),
            "cos": np.asarray(cos, np.float32),
            "sin": np.asarray(sin, np.float32),
            "maskl": maskl,
            "ident": ident,
            "ones": ones,
        })
    return in_maps


def _combine(results):
    outs = []
    for b in range(2):
        acc = results[4 * b]["outT"].astype(np.float32)
        for hg in range(1, 4):
            acc = acc + results[4 * b + hg]["outT"].astype(np.float32)
        outs.append(acc.T)
    return np.ascontiguousarray(np.stack(outs))


def kernel(x, cos, sin, Wqkv, Wproj):
    nc = _get_program()
    in_maps = _make_in_maps(np.asarray(x, np.float32), cos, sin,
                            np.asarray(Wqkv, np.float32), np.asarray(Wproj, np.float32))
    res = run_bass_kernel_spmd(nc, in_maps, list(range(8)))
    return _combine(res.results)


def _install_ntff_shim():
    """Provide the antenv.axon_hooks registry this container lacks, wired to
    the ctypes NTFF hook from trn_agent_boot, so trace=True works."""
    import types

    if "antenv.axon_hooks" in sys.modules:
        return
    hook = None
    try:
        from trn_agent_boot.trn_boot import _ntff_profile_via_ctypes
        hook = _ntff_profile_via_ctypes("/opt/axon/libaxon_pjrt.so")
    except Exception as e:
        print("ntff shim unavailable:", e)
    mod = types.ModuleType("antenv.axon_hooks")
    mod._hook = hook
    mod.get_axon_ntff_profile_hook = lambda: mod._hook
    mod.set_axon_ntff_profile_hook = lambda h: setattr(mod, "_hook", h)
    sys.modules["antenv.axon_hooks"] = mod
    # keep artifacts local; the bucket upload path isn't available here
    import concourse.bass_utils as bu
    bu.upload_artifacts = lambda tmpdir: tmpdir


def kernel_profiled(x, cos, sin, Wqkv, Wproj, trace_cores=None, tmpdir=None):
    nc = _get_program()
    _install_ntff_shim()
    in_maps = _make_in_maps(np.asarray(x, np.float32), cos, sin,
                            np.asarray(Wqkv, np.float32), np.asarray(Wproj, np.float32))
    res = run_bass_kernel_spmd(nc, in_maps, list(range(8)), trace=True,
                               trace_cores=trace_cores, tmpdir=tmpdir)
    return _combine(res.results), res
